# revision 29
# baseline (speedup 1.0000x reference)
"""AdaptiveSudokuLoss on 8 TRN2 NeuronCores — pure data-parallel.

Full inputs: outputs (65536, 81, 9) f32, targets (65536, 81) int64.
Output: scalar f32 loss.

Host preprocessing: cast x to fp16, pad digit axis 9 -> 10 with -100
(exp -> 0; keeps every run even-length/4B-aligned so fp16 tensor_tensor
hits the DVE 2x packed mode); targets are sent as onehot {0,1} fp16.

Math per cell (9 logits x_d):
  e = exp(x); Z = sum_d e; logZ = ln Z; p = e * exp(-ln Z)
  loss = (1.1*S_logZ - S_xt - 0.1*S_px)/N + 0.5*(S_r+S_c+S_b)/(B*9*27)

Engine split per tile (vs the all-DVE/ScalarE baseline):
  ScalarE: exp, ln(+accum S_logZ), r=exp(-lnZ), r10 broadcast, 3x
           Square(g-1)+accum (rows/cols from PSUM, boxes too)
  DVE:     Z via 4 packed 2x tensor_tensor adds (not 1x tensor_reduce),
           p = e*r10, and two fused scalar_tensor_tensor passes that
           compute x*p / x*onehot AND their full sums (accum_out) in one
           2x pass each — replacing is_equal + 2 mults + 2 ScalarE
           accumulation passes of the baseline.
  PE:      all 27 constraint group sums (rows/cols/boxes) as
           identity-weight matmuls accumulating in PSUM — the TensorE
           was 100%% idle in the baseline while DVE did these adds.

Each core processes 8192 samples, emits partial sums as [128, 8] f32;
host combines. No collectives.
"""
import numpy as np

import concourse.bass as bass
import concourse.bass_utils as bass_utils_mod
import concourse.tile as tile_mod
from concourse import mybir
from concourse.bass_utils import run_bass_kernel_spmd
from concourse.masks import make_identity
from concourse.vector_clock import ScopedClock

# (walrus --enable-ldw-opt=true was tried to dedupe the ~600 identical
# LDWEIGHTS, but this b16 build fails codegen on standalone InstLdweights.)

# ---------------------------------------------------------------- tile fix --
# walrus (b16 2026-05-04) accepts only one sem-wait per instruction; Tile's
# add_semaphores attaches several. Hoist extras onto same-engine NOPs.

_nop_counter = [0]


def _split_multi_waits(nc):
    for fn in nc.m.functions:
        for bb in fn.blocks:
            out = []
            changed = False
            for inst in bb.instructions:
                si = inst.sync_info
                if si is not None and len(si.on_wait) > 1:
                    waits = list(si.on_wait)
                    for w in waits[:-1]:
                        _nop_counter[0] += 1
                        n = mybir.InstNoOp(
                            name=f"I-waitsplit-{_nop_counter[0]}", ins=[], outs=[])
                        n.engine = inst.engine
                        n.sync_info = mybir.SyncInfo(on_wait=[w], on_update=[])
                        out.append(n)
                    si.on_wait = waits[-1:]
                    inst.sync_info = si
                    changed = True
                out.append(inst)
            if changed:
                bb.instructions = out


def _patched_drain_and_barrier(self, tick_clock, wait_clock):
    nc = self.nc
    probe = nc.sync.nop()
    wait_clock.add_sem_waits(probe.ins, ScopedClock({None: tick_clock.global_clock}))
    nc.sync.drain()
    nc.all_engine_barrier()
    assert self.sems is not None
    popped = nc._tile_sem_poison_stack.pop()
    assert popped is self._sem_poison
    nc.clear_and_free_semaphores(list(self.sems.allocated().values()))
    nc.all_engine_barrier()
    _split_multi_waits(nc)


tile_mod.TileContext._drain_and_barrier = _patched_drain_and_barrier

# ------------------------------------------------------------------- consts --
B = 65536
NCORES = 8
BS = B // NCORES            # samples per core = 8192
P = 128                     # partitions
SPP = BS // P               # samples per partition = 64
CPP = SPP * 81              # cells per partition = 5184
D = 10                      # padded digit axis
FPP = CPP * D               # fp16 elems per partition = 51840
NT = 8                      # tiles
TS = SPP // NT              # samples per partition per tile = 8
TC = TS * 81                # cells = 648
TF = TC * D                 # elems = 6480

F32 = mybir.dt.float32
F16 = mybir.dt.float16
ALU = mybir.AluOpType
ACTF = mybir.ActivationFunctionType
AX = mybir.AxisListType

_CACHE = {}


def _build():
    nc = bass.Bass()
    cm1 = nc.alloc_sbuf_tensor("const-float32-neg1", [128, 1], F32)
    nc.gpsimd.memset(cm1.ap(), -1.0)
    nc.const_aps.aps[(F32, -1.0)] = cm1.ap()
    ident = nc.alloc_sbuf_tensor("identity-f16", [128, 128], F16)
    make_identity(nc, ident.ap())
    ones = nc.alloc_sbuf_tensor("ones-f16", [128, 1], F16)
    nc.gpsimd.memset(ones.ap(), 1.0)
    nc.all_engine_barrier()
    x_ext = nc.declare_dram_parameter("x", [P, FPP], F16, isOutput=False)
    t_ext = nc.declare_dram_parameter("t", [P, FPP], F16, isOutput=False)
    out_ext = nc.declare_dram_parameter("out", [P, 8], F32, isOutput=True)
    idap = ident.ap()
    onesap = ones.ap()

    with tile_mod.TileContext(nc) as tc:
        with (
            tc.tile_pool(name="work", bufs=3) as wp,
            tc.tile_pool(name="deep", bufs=4) as dp,
            tc.tile_pool(name="pers", bufs=1) as pp,
            tc.tile_pool(name="psum", bufs=1, space="PSUM") as qp,
        ):
            # wp: per-iter 2x(xt,oht full) + 2 halves of small chain tiles
            # dp: pt (PE reads it latest -> 3 bufs of slack)
            accL = pp.tile([P, 2 * NT], F32)   # sum logZ (per half-tile)
            # running sum of x*(p + 10*onehot), PE-accumulated in PSUM
            xq_ps = qp.tile([1, 512], F32)
            accR = pp.tile([P, NT], F32)   # sum (g-1)^2 rows
            accC = pp.tile([P, NT], F32)   # cols
            accB = pp.tile([P, NT], F32)   # boxes

            def emit_squares(prev):
                # sum (g-1)^2 per type on ScalarE: Square(g + (-1)), accum.
                # Deferred one iteration so the PSUM WAR clears before the
                # next tile's matmuls need the banks.
                kk, rows_ps, cols_ps, box_ps = prev
                sqs = wp.tile([P, TC], F16)
                sqv = sqs[:].rearrange("p (s g) -> p s g", g=81)
                nc.scalar.activation(sqv, rows_ps[:, :, 0:81], ACTF.Square,
                                     bias=-1.0, accum_out=accR[:, kk:kk + 1])
                nc.scalar.activation(sqv, cols_ps[:, :, 0:81], ACTF.Square,
                                     bias=-1.0, accum_out=accC[:, kk:kk + 1])
                sqb = sqs[:].rearrange("p (R s g) -> p R s g", R=3, g=27)
                nc.scalar.activation(sqb, box_ps[:, :, :, 0:27], ACTF.Square,
                                     bias=-1.0, accum_out=accB[:, kk:kk + 1])

            prev = None
            for k in range(NT):
                xt = wp.tile([P, TF], F16)
                nc.sync.dma_start(xt[:], x_ext[:, k * TF:(k + 1) * TF])
                oht = wp.tile([P, TF], F16)
                nc.sync.dma_start(oht[:], t_ext[:, k * TF:(k + 1) * TF])

                if prev is not None:
                    emit_squares(prev)

                # Front-end (exp -> Z -> ln -> 1/Z -> broadcast -> p -> q ->
                # x*q) runs at HALF-tile granularity: the serial chain's
                # latency halves and the PE can start on half 0 while
                # half 1 is still in the scalar/vector pipe.
                HC = TC // 2            # cells per half = 324
                HF = HC * D             # elems per half = 3240
                pt = dp.tile([P, TF], F16)
                p6 = pt[:].rearrange("p (s r c d) -> p s r c d",
                                     s=TS, r=9, c=9, d=D)
                p8 = pt[:].rearrange("p (s R i C j d) -> p s R i C j d",
                                     s=TS, R=3, i=3, C=3, j=3, d=D)
                rows_ps = qp.tile([P, TS, 128], F32)
                cols_ps = qp.tile([P, TS, 128], F32)
                box_ps = qp.tile([P, 3, TS, 32], F32)

                for h in (0, 1):
                    fsl = slice(h * HF, (h + 1) * HF)
                    et = wp.tile([P, HF], F16)
                    nc.scalar.activation(et[:], xt[:, fsl], ACTF.Exp)
                    e3 = et[:].rearrange("p (c d) -> p c d", d=D)

                    # Z per cell via packed 2x adds (tensor_reduce is 1x):
                    # t1[j] = e[j]+e[j+4] (j=0..3); t1[0:2] += e[8:10];
                    # t1[0:2] += t1[2:4]; Z = t1[0]+t1[1]
                    t1 = wp.tile([P, HC * 4], F16)
                    t1v = t1[:].rearrange("p (c f) -> p c f", f=4)
                    nc.vector.tensor_tensor(t1v, e3[:, :, 0:4], e3[:, :, 4:8],
                                            op=ALU.add)
                    nc.vector.tensor_tensor(t1v[:, :, 0:2], t1v[:, :, 0:2],
                                            e3[:, :, 8:10], op=ALU.add)
                    nc.vector.tensor_tensor(t1v[:, :, 0:2], t1v[:, :, 0:2],
                                            t1v[:, :, 2:4], op=ALU.add)
                    st = wp.tile([P, HC], F32)
                    nc.vector.tensor_tensor(st[:].unsqueeze(2),
                                            t1v[:, :, 0:1], t1v[:, :, 1:2],
                                            op=ALU.add)

                    lst = wp.tile([P, HC], F32)
                    nc.scalar.activation(lst[:], st[:], ACTF.Ln,
                                         accum_out=accL[:, 2 * k + h:
                                                        2 * k + h + 1])
                    rt = wp.tile([P, HC], F16)
                    nc.scalar.activation(rt[:], lst[:], ACTF.Exp, scale=-1.0)
                    # broadcast r only x2 (cheap), then p = e*r as five
                    # d-pair TTs — every view stays 4B-aligned/even so DVE
                    # keeps 2x, and ScalarE sheds the big x10 broadcast.
                    r2 = wp.tile([P, HC * 2], F16)
                    r2v = r2[:].rearrange("p (c t) -> p c t", t=2)
                    nc.scalar.activation(
                        r2v, rt[:].unsqueeze(2).broadcast_to([P, HC, 2]),
                        ACTF.Copy)
                    # single 2x TT: r2 rides a stride-0 middle dim (c, j, t);
                    # the innermost [1,2] keeps the packed mode.
                    p5 = pt[:, fsl].rearrange("p (c j t) -> p c j t",
                                              j=D // 2, t=2)
                    e5 = et[:].rearrange("p (c j t) -> p c j t",
                                         j=D // 2, t=2)
                    r5 = r2v.unsqueeze(2).broadcast_to([P, HC, D // 2, 2])
                    nc.vector.tensor_tensor(p5, e5, r5, op=ALU.mult)

                    # q = p + 10*onehot; S_xq = sum x*q = S_px + 10*S_xt.
                    # Two 2x TTs; the sum runs on the PE (ones-vector
                    # matmuls accumulating every half into one PSUM bank).
                    # q lands in-place over onehot, the product over e.
                    qt = oht[:, fsl]
                    nc.vector.tensor_tensor(qt, pt[:, fsl], oht[:, fsl],
                                            op=ALU.add)
                    scr = et
                    nc.vector.tensor_tensor(scr[:], xt[:, fsl], qt,
                                            op=ALU.mult)
                    for c in range(7):
                        n = 512 if c < 6 else HF - 6 * 512
                        nc.tensor.matmul(
                            xq_ps[0:1, 0:n], onesap,
                            scr[:, 512 * c:512 * c + n],
                            start=(k == 0 and h == 0 and c == 0),
                            stop=(k == NT - 1 and h == 1 and c == 6))

                    # rows/cols group sums for this half's 4 samples:
                    # identity matmuls accumulating in PSUM.
                    ssl = slice(4 * h, 4 * h + 4)
                    for c in range(9):
                        nc.tensor.matmul(
                            rows_ps[:, ssl, 0:81], idap, p6[:, ssl, :, c, 0:9],
                            start=(c == 0), stop=(c == 8))
                        nc.tensor.matmul(
                            cols_ps[:, ssl, 0:81], idap, p6[:, ssl, c, :, 0:9],
                            start=(c == 0), stop=(c == 8))

                # boxes: R regions are bank-aligned (1KB each) so no s-chunk
                for R in range(3):
                    for ij in range(9):
                        i, j = divmod(ij, 3)
                        nc.tensor.matmul(
                            box_ps[:, R, :, 0:27], idap,
                            p8[:, :, R, i, :, j, 0:9],
                            start=(ij == 0), stop=(ij == 8))
                prev = (k, rows_ps, cols_ps, box_ps)

            emit_squares(prev)

            ot = pp.tile([P, 8], F32)
            nc.vector.memset(ot[:, 1:3], 0.0)
            nc.vector.memset(ot[:, 6:8], 0.0)
            nc.vector.tensor_reduce(ot[:, 0:1], accL[:], axis=AX.X, op=ALU.add)
            nc.vector.tensor_reduce(ot[0:1, 1:2], xq_ps[:], axis=AX.X,
                                    op=ALU.add)
            nc.vector.tensor_reduce(ot[:, 3:4], accR[:], axis=AX.X, op=ALU.add)
            nc.vector.tensor_reduce(ot[:, 4:5], accC[:], axis=AX.X, op=ALU.add)
            nc.vector.tensor_reduce(ot[:, 5:6], accB[:], axis=AX.X, op=ALU.add)
            nc.sync.dma_start(out_ext[:], ot[:])
    return nc


def _get_nc():
    if "nc" not in _CACHE:
        _CACHE["nc"] = _build()
    return _CACHE["nc"]


def _prep_x(outputs):
    """(B, 81, 9) f32 -> per-core [128, FPP] fp16 with digit pad -100."""
    xb = np.full((B, 81, D), -100.0, dtype=np.float16)
    xb[:, :, :9] = outputs.astype(np.float16)
    return xb.reshape(NCORES, P, FPP)


def _prep_t(targets):
    """(B, 81) -> per-core [128, FPP] fp16 10*onehot (pad digit col = 0).

    The factor 10 folds the CE/conf coefficients: sum x*(p + 10*onehot)
    = S_px + 10*S_xt, and the loss needs -S_xt - 0.1*S_px = -0.1*that."""
    t = targets.reshape(NCORES, P, CPP)
    oh = (np.arange(D, dtype=t.dtype)[None, None, None, :] == t[..., None])
    oh10 = oh.astype(np.float16) * np.float16(10.0)
    return np.ascontiguousarray(oh10.reshape(NCORES, P, FPP))


def kernel(outputs: np.ndarray, targets: np.ndarray, _want_results=False,
           **run_kwargs) -> np.ndarray:
    nc = _get_nc()
    xs_all = _prep_x(np.ascontiguousarray(outputs, dtype=np.float32))
    ts_all = _prep_t(np.ascontiguousarray(targets))
    in_maps = [{"x": xs_all[i], "t": ts_all[i]} for i in range(NCORES)]
    res = run_bass_kernel_spmd(nc, in_maps, core_ids=list(range(NCORES)),
                               **run_kwargs)

    S = np.zeros(8, dtype=np.float64)
    for i in range(NCORES):
        S += res.results[i]["out"].astype(np.float64).sum(axis=0)
    S_logZ, S_xq, S_r, S_c, S_b = S[0], S[1], S[3], S[4], S[5]
    N = float(B * 81)
    term1 = (1.1 * S_logZ - 0.1 * S_xq) / N
    csum = S_r + S_c + S_b
    loss = term1 + 0.5 * csum / (B * 9.0 * 27.0)
    out = np.asarray(loss, dtype=np.float32)
    if _want_results:
        return out, res
    return out
